# revision 1
# baseline (speedup 1.0000x reference)
"""Causal self-attention (B=4, S=2048, D=1024, H=16) on 8 trn2 cores.

Sharding: core c -> (batch b = c//2, head-half hh = c%2). Each core:
  - computes Q/K/V projections for its batch restricted to its 8 heads
    (512 of the 1024 feature columns),
  - runs causal attention for those heads,
  - computes a partial out-projection part = attnO @ w_o[rows of its heads].
Host: out[b] = part[2b] + part[2b+1] + (b_v @ w_o + b_o), and ships x
pre-transposed (feature-major) per core so no on-device transposes are
needed. (The V bias contributes b_v @ w_o to the output because softmax
rows sum to 1; the K bias is dropped entirely -- it only shifts scores
by a per-query constant, which softmax cancels.)

On-core layouts:
  xT   [1024,2048]  feature-major x, DMA'd directly (8 tiles [128,2048])
  qt/kt[512 ,2048]  feature-major bf16; tile g holds heads 2g,2g+1
  v_aug[2048, 520]  token-major bf16, 65 cols/head: 64 V cols + a ones
                    column (makes the PV matmul also produce the softmax
                    denominator as PSUM row 64)
  scores ST [nk,mq] per 128-row key tile; exp on ACT (scale=1/8, no max
                    subtraction -- scores are ~N(0,1), exp is safe)
  causal mask       affine_select (GPSIMD) zeroes exp(masked) entries on
                    the 128-column diagonal boundary only; fully-masked
                    column prefixes are skipped in score/exp/PV entirely
  normalization     reciprocal of sum row (DVE) + PE ones-broadcast
                    (dedicated PSUM pool) + DVE mul

Scheduling: the PE stream is hand-interleaved so attention (which paces
on ACT exp) always has single-matmul projection/out-projection filler
units between score/PV tiles (cycle-weighted zip); PV trails scores by
one tile so it never waits on exp; normalization is split into a DVE
prefix (right after the last PV) and a deferred PE broadcast.

Projections and the out-projection run in float32r (~1e-4 rounding);
scores run in bf16 (q/k rounded once after projection), and exp/V are
bf16 through the P@V stage: end-to-end max rel err ~2e-3 vs fp32.
"""

import sys

if "/opt/trn_rl_repo" not in sys.path:
    sys.path.insert(0, "/opt/trn_rl_repo")

import numpy as np

import concourse.bass as bass
import concourse.tile as tile
from concourse import bacc, mybir
from concourse.bass_utils import run_bass_kernel_spmd
from concourse.masks import make_identity

N_CORES = 8
S = 2048
D = 1024
DH = 512          # per-core feature width (8 heads x 64)
HD = 64           # head dim
NH_LOC = 8        # heads per core
F32 = mybir.dt.float32
F32R = mybir.dt.float32r
BF16D = mybir.dt.bfloat16
EXP = mybir.ActivationFunctionType.Exp
GE = mybir.AluOpType.is_ge

_PROGRAM = None


def _build_program(n_repeat=1):
    nc = bacc.Bacc("TRN2", target_bir_lowering=False, debug=False,
                   num_devices=N_CORES)
    # x arrives pre-transposed (feature-major) from the host shard prep;
    # this removes all on-device transposes and their PSUM/DVE traffic.
    x_d = nc.dram_tensor("x", [D, S], F32, kind="ExternalInput").ap()
    wq_d = nc.dram_tensor("wq", [D, DH], F32, kind="ExternalInput").ap()
    wk_d = nc.dram_tensor("wk", [D, DH], F32, kind="ExternalInput").ap()
    wv_d = nc.dram_tensor("wv", [D, DH], F32, kind="ExternalInput").ap()
    wo_d = nc.dram_tensor("wo", [DH, D], F32, kind="ExternalInput").ap()
    bq_d = nc.dram_tensor("bq", [DH], F32, kind="ExternalInput").ap()
    part_d = nc.dram_tensor("part", [S, D], F32, kind="ExternalOutput").ap()

    with tile.TileContext(nc) as tc:
        for _ in range(n_repeat):
            _emit(nc, tc, x_d, wq_d, wk_d, wv_d, wo_d, bq_d, part_d)
    nc.compile()
    return nc


def _emit(nc, tc, x_d, wq_d, wk_d, wv_d, wo_d, bq_d, part_d):
    """Emission is hand-pipelined: per-engine instruction order follows
    emission order, so work is zipped so the PE always has filler matmuls
    queued behind attention iterations that pace on the ACT engine:

      [transpose x (PE) || V projection || wv/wq/wk DMAs]
      [Q0/K0 projection]
      [attention pair0 || Q1/K1 projection]   (4 attn iters : 1 proj unit)
      [attention pair1 || Q2/K2 projection]
      [attention pair2 || Q3/K3 projection]
      [attention pair3 || per-chunk out-projection]

    Q/K tiles rotate through 2 slots per tag (pair g is dead once its
    attention is done), which is what makes everything fit in SBUF.
    """
    from contextlib import ExitStack

    BF16 = mybir.dt.bfloat16
    es = ExitStack()
    with es:
        const_pool = es.enter_context(tc.tile_pool(name="const", bufs=1))
        v_pool = es.enter_context(tc.tile_pool(name="vaug", bufs=1))
        qkv_pool = es.enter_context(tc.tile_pool(name="qkv", bufs=2))
        exp_pool = es.enter_context(tc.tile_pool(name="ex", bufs=6))
        rc_pool = es.enter_context(tc.tile_pool(name="rc", bufs=4))
        # Chain PSUM (projection/out-projection accumulators) and the
        # normalize broadcast get SEPARATE pools: norm_b is interleaved
        # into open projection chains, and sharing a rotating pool with
        # them serializes the PE stream on hardware semaphores.
        pp_pool = es.enter_context(
            tc.tile_pool(name="pp", bufs=1, space="PSUM"))
        bc_pool = es.enter_context(
            tc.tile_pool(name="bcp", bufs=1, space="PSUM"))

        ones_bf = const_pool.tile([128, NH_LOC], BF16, name="ones_bf",
                                  tag="ones")
        nc.vector.memset(ones_bf, 1.0)
        ones_f32 = const_pool.tile([128, HD], F32, name="ones_f32",
                                   tag="onesf")
        nc.vector.memset(ones_f32, 1.0)
        # Warm the ACT exp table during phase A; otherwise the first
        # attention exp pays the 1.3us table load on the critical path.
        warm = const_pool.tile([1, 1], F32, name="warm", tag="warm")
        nc.scalar.activation(warm[:], ones_f32[0:1, 0:1], EXP, scale=1.0)
        ones_fr = const_pool.tile([128, HD], F32R, name="ones_fr",
                                  tag="onesfr")
        nc.vector.tensor_copy(ones_fr[:], ones_f32[:])
        bq_sb = const_pool.tile([128, 4], F32, name="bq_sb", tag="bq")

        v_aug = [v_pool.tile([128, NH_LOC * (HD + 1)], BF16,
                             name=f"va{t}", tag=f"va{t}")
                 for t in range(16)]

        xt_cm = tc.tile_pool(name="xtp", bufs=1, side="right")
        xt_pool = xt_cm.__enter__()
        xt = [xt_pool.tile([128, S], F32R, name=f"xt{k}", tag=f"xt{k}")
              for k in range(8)]
        # Persistent Q/K projection weights, one wide [128, 512] tile per
        # contraction chunk covering all four head pairs (batched DMAs).
        w_cm = tc.tile_pool(name="wp", bufs=1, side="right")
        wq_pool = w_cm.__enter__()
        wq_t = []
        wk_t = []

        qt = {}
        kt = {}
        ot_map = {}

        def emit_proj_units(g, pool):
            """Q/K projection for pair g as a list of emission closures.

            qt/kt are bf16: the score matmuls run at 1.0 PE cycles/row for
            any moving width (f32r falls to 4.0 below 256), and the compiler
            can split bf16 stationaries into pipelined ldweights+matmul.
            K gets no bias: (q+bq)@(k+bk) differs from (q+bq)@k only by a
            per-query constant, which softmax cancels.
            """
            units = []

            def alloc_out(which):
                t = qkv_pool.tile([128, S], BF16,
                                  name=f"{which}t{g}", tag=which)
                (qt if which == "qt" else kt)[g] = t
                return t

            state = {}

            def mk_kc(which, mc, kc):
                # Single-matmul filler granularity (~210ns of PE) so the
                # zip can land one between every pair of attention tiles,
                # covering the per-tile ACT-overhead deficit.
                def u():
                    if which[:2] not in state:
                        state[which[:2]] = alloc_out(which)
                    wt = wq_t if which == "qt" else wk_t
                    if kc == 0:
                        state["pp"] = pool.tile(
                            [128, 512], F32,
                            name=f"pp{which}{g}_{mc}", tag="pp")
                    nc.tensor.matmul(
                        state["pp"][:],
                        wt[kc][:, g * 128:(g + 1) * 128],
                        xt[kc][:, mc * 512:(mc + 1) * 512],
                        start=(kc == 0), stop=(kc == 7))
                    if kc == 7:
                        out_t = state[which[:2]]
                        pp = state.pop("pp")
                        if which == "qt":
                            nc.vector.tensor_scalar_add(
                                out_t[:, mc * 512:(mc + 1) * 512],
                                pp[:], bq_sb[:, g:g + 1])
                        else:
                            nc.vector.tensor_copy(
                                out_t[:, mc * 512:(mc + 1) * 512], pp[:])
                return u

            for which in ("qt", "kt"):
                for mc in range(4):
                    for kc in range(8):
                        units.append((mk_kc(which, mc, kc), 512))
            return units

        # ---- Phase A: stream xT + wv in, V projection zipped ----------
        # ppb is a phase-scoped 2-buf chain pool: the back-to-back V-proj
        # and Q0/K0 chains overlap accumulation with the previous chain's
        # PSUM drain; it closes before the attention PSUM pools open.
        with (
            tc.tile_pool(name="wv", bufs=8, side="right") as wv_pool,
            tc.tile_pool(name="ppb", bufs=2, space="PSUM") as ppb_pool,
        ):
            # Interleave wv[kc] with the first 128 xT columns of chunk kc:
            # the V-proj kc-chain for token tile 0 can then start ~1us in
            # and track the DMA stream instead of waiting for all 2.5MB.
            wtv = []
            for kc in range(8):
                w_t = wv_pool.tile([128, DH], F32R, name=f"wv{kc}", tag="wv")
                nc.sync.dma_start(
                    out=w_t,
                    in_=wv_d[kc * 128:(kc + 1) * 128, :].bitcast(F32R))
                wtv.append(w_t)
                nc.sync.dma_start(
                    out=xt[kc][:, 0:128],
                    in_=x_d[kc * 128:(kc + 1) * 128, 0:128].bitcast(F32R))
            for kc in range(8):
                nc.sync.dma_start(
                    out=xt[kc][:, 128:512],
                    in_=x_d[kc * 128:(kc + 1) * 128, 128:512].bitcast(F32R))
            for mg in range(1, 4):
                cs = slice(mg * 512, (mg + 1) * 512)
                for kc in range(8):
                    nc.sync.dma_start(
                        out=xt[kc][:, cs],
                        in_=x_d[kc * 128:(kc + 1) * 128, cs].bitcast(F32R))
            for g in range(4):
                sl = slice(g * 128, (g + 1) * 128)
                nc.sync.dma_start(
                    out=bq_sb[:, g:g + 1],
                    in_=bq_d[sl].rearrange("(p one) -> p one", one=1))
            for mt in range(16):
                pp = ppb_pool.tile([128, 512], F32,
                                  name=f"ppv{mt}", tag="pp")
                for kc in range(8):
                    nc.tensor.matmul(
                        pp[:],
                        xt[kc][:, mt * 128:(mt + 1) * 128],
                        wtv[kc][:],
                        start=(kc == 0), stop=(kc == 7))
                va3 = v_aug[mt].rearrange("p (h c) -> p h c", h=NH_LOC)
                nc.vector.tensor_copy(
                    va3[:, :, 0:HD],
                    pp[:].rearrange("p (h c) -> p h c", h=NH_LOC))
                nc.vector.tensor_copy(
                    va3[:, :, HD:HD + 1],
                    ones_bf[:, 0:NH_LOC].rearrange(
                        "p (h one) -> p h one", one=1))
            # Q/K weights for all four pairs in eight wide transfers each;
            # they queue behind the x tiles so they don't starve the
            # transposes, and land just before the first Q/K projection.
            for kc in range(8):
                w_t = wq_pool.tile([128, DH], F32R, name=f"wqa{kc}",
                                   tag=f"wq{kc}")
                nc.sync.dma_start(
                    out=w_t,
                    in_=wq_d[kc * 128:(kc + 1) * 128, :].bitcast(F32R))
                wq_t.append(w_t)
            for kc in range(8):
                w_t = wq_pool.tile([128, DH], F32R, name=f"wka{kc}",
                                   tag=f"wk{kc}")
                nc.sync.dma_start(
                    out=w_t,
                    in_=wk_d[kc * 128:(kc + 1) * 128, :].bitcast(F32R))
                wk_t.append(w_t)
            # Q0/K0 inside the phase-A scope so its chains get the 2-buf
            # pool before it closes.
            for u, _ in emit_proj_units(0, ppb_pool):
                u()

        # ---- attention-phase pools (open after ppb frees its banks) ----
        st_pool = es.enter_context(
            tc.tile_pool(name="st", bufs=2, space="PSUM"))
        otp_pool = es.enter_context(
            tc.tile_pool(name="ops", bufs=2, space="PSUM"))
        ot_pool = es.enter_context(tc.tile_pool(name="otl", bufs=16))

        ex_map = {}

        def emit_scores_exp(g, j, t):
            # Columns [0, z) of a diagonal tile are fully masked: skip them
            # in the score matmul, exp, mask, and PV accumulation entirely
            # (the PV start=True tile always covers the full width, so the
            # untouched PSUM columns keep their accumulated values).
            d = t - 4 * j
            z = 0 if d < 0 else 128 * d
            w = 512 - z
            mq = slice(j * 512 + z, (j + 1) * 512)
            nk = slice(t * 128, (t + 1) * 128)
            st = st_pool.tile([128, 1024], F32,
                              name=f"st{j}_{g}_{t}", tag="st")
            for hl in range(2):
                dsl = slice(hl * 64, hl * 64 + 64)
                nc.tensor.matmul(
                    st[:, hl * 512:hl * 512 + w],
                    kt[g][dsl, nk], qt[g][dsl, mq],
                    start=True, stop=True)
            ex = exp_pool.tile([128, 1024], BF16,
                               name=f"ex{j}_{g}_{t}", tag="ex")
            if d < 0:
                # off-diagonal: both head halves in one wide ACT op
                nc.scalar.activation(ex[:, 0:1024], st[:, 0:1024],
                                     EXP, scale=0.125)
            else:
                # diagonal: both head halves in one strided 3D op each for
                # exp and mask (the head dim is a stride-512 middle axis;
                # the mask predicate ignores it via a 0-step pattern pair)
                st3 = st.rearrange("p (h q) -> p h q", h=2)[:, :, 0:w]
                ex3 = ex.rearrange("p (h q) -> p h q", h=2)[:, :, 0:w]
                nc.scalar.activation(ex3, st3, EXP, scale=0.125)
                # keep where local_mq >= local_nk: y - p >= 0. Only the
                # first 128 columns straddle the diagonal; beyond them
                # local_mq >= 128 > any local_nk, so they pass untouched.
                exb = ex.rearrange("p (h q) -> p h q", h=2)[:, :, 0:128]
                nc.gpsimd.affine_select(
                    out=exb, in_=exb,
                    compare_op=GE, fill=0.0, base=0,
                    channel_multiplier=-1,
                    pattern=[[0, 2], [1, 128]])
            ex_map[(g, j, t)] = ex

        def emit_pv(g, j, t, t_max):
            # Issued one tile behind scores/exp so the PE never waits on
            # the ACT engine's exp of the same tile.
            d = t - 4 * j
            z = 0 if d < 0 else 128 * d
            w = 512 - z
            ex = ex_map.pop((g, j, t))
            o_ps = ot_map[("ps", g, j)]
            for hl in range(2):
                h = 2 * g + hl
                nc.tensor.matmul(
                    o_ps[hl][:, z:512],
                    v_aug[t][:, 65 * h:65 * h + 65],
                    ex[:, hl * 512:hl * 512 + w],
                    start=(t == 0), stop=(t == t_max - 1))

        def emit_norm_a(g, j):
            # DVE-only prefix of the normalization: copy PSUM out (frees
            # the accumulation banks) and invert the sum rows. Emitted
            # immediately after the last PV so the DVE chain runs while
            # the PE continues with the next chunk's scores.
            o_ps = ot_map.pop(("ps", g, j))
            ocps = []
            for hl in range(2):
                ocp = rc_pool.tile([65, 512], F32R,
                                   name=f"ocp{j}_{g}_{hl}", tag="ocp")
                nc.vector.tensor_copy(ocp[:], o_ps[hl][:])
                with nc.allow_low_precision(reason="f32r recip row"):
                    nc.vector.reciprocal(ocp[64:65, :], ocp[64:65, :])
                ocps.append(ocp)
            ot_map[("ocp", g, j)] = ocps

        def emit_norm_b(g, j):
            # Deferred PE part: broadcast 1/sum across partitions with a
            # rank-1 matmul (from the dedicated bc PSUM pool), then scale.
            ocps = ot_map.pop(("ocp", g, j))
            ot_t = ot_pool.tile([128, 512], F32R,
                                name=f"ot{j}_{g}", tag="ot")
            ot_map[(j, g)] = ot_t
            for hl in range(2):
                ocp = ocps[hl]
                bc = bc_pool.tile([64, 512], F32,
                                  name=f"bc{j}_{g}_{hl}", tag="bc")
                nc.tensor.matmul(bc[:], ones_fr[64:65, 0:HD],
                                 ocp[64:65, :], start=True, stop=True)
                nc.vector.tensor_mul(
                    ot_t[64 * hl:64 * hl + 64, :],
                    ocp[0:64, :], bc[:])

        def att_unit_groups(g, carry_in):
            """Per-chunk unit lists for pair g; returns (groups, carry_out).

            Each chunk j: [alloc, se(0), <deferred norm_b>, se(1), pv(0),
            ..., se(last), pv(last-1), pv(last), norm_a(j)].
            norm_b(j) lands after se(j+1, 0) so its broadcast matmul never
            stalls the PE on the reciprocal chain; the last chunk's norm_b
            is carried into the next pair (or the caller's tail).
            """
            groups = []
            carry = carry_in
            for j in range(4):
                t_max = 4 * (j + 1)

                def mk(f, *a):
                    return lambda: f(*a)

                def wof(t, j=j):
                    d = t - 4 * j
                    return 2 * (512 - (0 if d < 0 else 128 * d))

                def mk_alloc(g=g, j=j):
                    def u():
                        ot_map[("ps", g, j)] = [
                            otp_pool.tile([65, 512], F32,
                                          name=f"o{j}_{g}_{hl}", tag="ops")
                            for hl in range(2)]
                    return u

                units = [(mk_alloc(), 0),
                         (mk(emit_scores_exp, g, j, 0), wof(0))]
                if carry is not None:
                    units.append((carry, 1024))
                    carry = None
                for t in range(1, t_max):
                    units.append((mk(emit_scores_exp, g, j, t), wof(t)))
                    units.append((mk(emit_pv, g, j, t - 1, t_max),
                                  wof(t - 1)))
                units.append((mk(emit_pv, g, j, t_max - 1, t_max),
                              wof(t_max - 1)))
                units.append((mk(emit_norm_a, g, j), 0))
                carry = mk(emit_norm_b, g, j)
                groups.append(units)
            return groups, carry

        def zip_emit(primary, filler):
            """Interleave filler units into the primary stream, spreading
            them uniformly over the primary's PE-cycle timeline (both are
            lists of (closure, pe_cycles))."""
            tot_p = sum(c for _, c in primary) or 1
            tot_f = sum(c for _, c in filler) or 1
            n_f = len(filler)
            fi = 0
            cum_p = 0
            cum_f = 0
            for u, c in primary:
                u()
                cum_p += c
                while fi < n_f and cum_f * tot_p <= cum_p * tot_f:
                    fu, fc = filler[fi]
                    fu()
                    cum_f += fc
                    fi += 1
            while fi < n_f:
                filler[fi][0]()
                fi += 1

        # Attention(g) zipped with projections(g+1); Q0/K0 already done.
        carry = None
        for g in range(3):
            groups, carry = att_unit_groups(g, carry)
            zip_emit([u for grp in groups for u in grp],
                     emit_proj_units(g + 1, pp_pool))

        # xT and the projection weights are dead now; free them (they live
        # on the right-side SBUF stack, popped LIFO: wp then xtp) before the
        # out-projection pools open so the SBUF budget holds.
        w_cm.__exit__(None, None, None)
        xt_cm.__exit__(None, None, None)

        wo_pool = es.enter_context(tc.tile_pool(name="wo", bufs=4))
        os_pool = es.enter_context(tc.tile_pool(name="os", bufs=4))
        wo_t = []
        for fc in range(4):
            w_t = wo_pool.tile([128, D], F32R, name=f"wo{fc}", tag=f"wo{fc}")
            nc.sync.dma_start(
                out=w_t,
                in_=wo_d[fc * 128:(fc + 1) * 128, :].bitcast(F32R))
            wo_t.append(w_t)

        def outproj_units(j):
            units = []
            for mt in range(4 * j, 4 * j + 4):
                for nck in range(2):
                    st8 = {}

                    def mk_g(j=j, mt=mt, nck=nck, g=0, st8=st8):
                        def u():
                            msl = slice((mt - 4 * j) * 128,
                                        (mt - 4 * j) * 128 + 128)
                            if g == 0:
                                st8["op"] = pp_pool.tile(
                                    [128, 512], F32,
                                    name=f"op{mt}_{nck}", tag="pp")
                            nc.tensor.matmul(
                                st8["op"][:],
                                ot_map[(j, g)][:, msl],
                                wo_t[g][:, nck * 512:(nck + 1) * 512],
                                start=(g == 0), stop=(g == 3))
                            if g == 3:
                                # PSUM->SBUF staging (DMA and GPSIMD
                                # cannot read PSUM); DVE has slack here.
                                op = st8.pop("op")
                                osb = os_pool.tile(
                                    [128, 512], F32,
                                    name=f"os{mt}_{nck}", tag="os")
                                nc.vector.tensor_copy(osb[:], op[:])
                                nc.sync.dma_start(
                                    out=part_d[
                                        mt * 128:(mt + 1) * 128,
                                        nck * 512:(nck + 1) * 512],
                                    in_=osb[:])
                        return u

                    for g in range(4):
                        units.append((mk_g(g=g), 512))
            return units

        # Last pair: chunk j-1's out-projection is zipped between chunk j's
        # attention iterations so the PE keeps filler work while the
        # normalize chain drains; chunk 3's out-projection closes the tail.
        groups, carry = att_unit_groups(3, carry)
        for j in range(4):
            zip_emit(groups[j], outproj_units(j - 1) if j > 0 else [])
        carry()  # norm_b(3, 3)
        for u, _ in outproj_units(3):
            u()


def _get_program():
    global _PROGRAM
    if _PROGRAM is None:
        _PROGRAM = _build_program()
    return _PROGRAM


_EXEC = None


def _get_executor():
    """Build the sharded PJRT executable once and reuse it across calls.

    Mirrors bass2jax.run_bass_via_pjrt's multi-core branch, but caches the
    jitted callable so repeat kernel() calls skip retracing/recompilation.
    Returns (fn, in_names, out_names, out_shapes). fn takes globally
    concatenated inputs (n_cores*dim0, ...) plus donated zero output
    buffers, and returns concatenated outputs.
    """
    global _EXEC
    if _EXEC is None:
        import jax
        from jax.experimental.shard_map import shard_map
        from jax.sharding import Mesh, PartitionSpec

        from concourse import bass2jax

        bass2jax.install_neuronx_cc_hook()
        nc = _get_program()
        part_name = (nc.partition_id_tensor.name
                     if nc.partition_id_tensor else None)
        in_names, out_names, out_avals = [], [], []
        for alloc in nc.m.functions[0].allocations:
            if not isinstance(alloc, mybir.MemoryLocationSet):
                continue
            name = alloc.memorylocations[0].name
            if alloc.kind == "ExternalInput":
                if name != part_name:
                    in_names.append(name)
            elif alloc.kind == "ExternalOutput":
                out_names.append(name)
                out_avals.append(jax.core.ShapedArray(
                    tuple(alloc.tensor_shape), mybir.dt.np(alloc.dtype)))
        n_params = len(in_names)
        all_in = tuple(in_names) + tuple(out_names)
        if part_name is not None:
            all_in = all_in + (part_name,)

        def _body(*args):
            operands = list(args)
            if part_name is not None:
                operands.append(bass2jax.partition_id_tensor())
            outs = bass2jax._bass_exec_p.bind(
                *operands,
                out_avals=tuple(out_avals),
                in_names=all_in,
                out_names=tuple(out_names),
                lowering_input_output_aliases=(),
                sim_require_finite=True,
                sim_require_nnan=True,
                nc=nc)
            return tuple(outs)

        devices = jax.devices()[:N_CORES]
        mesh = Mesh(np.asarray(devices), ("core",))
        n_bufs = n_params + len(out_names)
        mapped = shard_map(_body, mesh=mesh,
                           in_specs=(PartitionSpec("core"),) * n_bufs,
                           out_specs=(PartitionSpec("core"),) * len(out_names),
                           check_rep=False)
        fn = jax.jit(mapped,
                     donate_argnums=tuple(range(n_params, n_bufs)),
                     keep_unused=True)
        # Non-donating twin: lets a timing loop reuse device-resident
        # argument buffers across calls (we write every element of every
        # output, so uninitialized result buffers are fine).
        fn_nodonate = jax.jit(mapped, keep_unused=True)
        out_shapes = [tuple(a.shape) for a in out_avals]
        _EXEC = (fn, fn_nodonate, in_names, out_names, out_shapes, mesh)
    return _EXEC


def run_cores(in_maps):
    """Run the SPMD program on 8 cores via the cached executable."""
    fn, _, in_names, out_names, out_shapes = _get_executor()[:5]
    concat_in = [np.concatenate([in_maps[c][n] for c in range(N_CORES)],
                                axis=0) for n in in_names]
    zeros = [np.zeros((N_CORES * s[0],) + s[1:], np.float32)
             for s in out_shapes]
    outs = fn(*concat_in, *zeros)
    res = []
    for c in range(N_CORES):
        res.append({
            n: np.asarray(outs[i]).reshape((N_CORES,) + out_shapes[i])[c]
            for i, n in enumerate(out_names)})
    return res


def make_in_maps(x, w_q, b_q, w_k, b_k, w_v, b_v, w_o, b_o):
    in_maps = []
    for c in range(N_CORES):
        b, hh = divmod(c, 2)
        cols = slice(hh * DH, (hh + 1) * DH)
        in_maps.append({
            "x": np.ascontiguousarray(x[b].T),
            "wq": np.ascontiguousarray(w_q[:, cols]),
            "wk": np.ascontiguousarray(w_k[:, cols]),
            "wv": np.ascontiguousarray(w_v[:, cols]),
            "wo": np.ascontiguousarray(w_o[cols, :]),
            "bq": np.ascontiguousarray(b_q[cols]),
        })
    return in_maps


def combine(parts, b_v, w_o, b_o):
    corr = (b_v @ w_o + b_o).astype(np.float32)
    out = np.empty((4, S, D), dtype=np.float32)
    for b in range(4):
        out[b] = parts[2 * b] + parts[2 * b + 1] + corr
    return out


def kernel(x, w_q, b_q, w_k, b_k, w_v, b_v, w_o, b_o):
    x = np.asarray(x, dtype=np.float32)
    w_q = np.asarray(w_q, dtype=np.float32)
    b_q = np.asarray(b_q, dtype=np.float32)
    w_k = np.asarray(w_k, dtype=np.float32)
    b_k = np.asarray(b_k, dtype=np.float32)
    w_v = np.asarray(w_v, dtype=np.float32)
    b_v = np.asarray(b_v, dtype=np.float32)
    w_o = np.asarray(w_o, dtype=np.float32)
    b_o = np.asarray(b_o, dtype=np.float32)

    in_maps = make_in_maps(x, w_q, b_q, w_k, b_k, w_v, b_v, w_o, b_o)
    res = run_cores(in_maps)
    parts = [res[c]["part"] for c in range(N_CORES)]
    return combine(parts, b_v, w_o, b_o)

